# revision 44
# baseline (speedup 1.0000x reference)
"""Bass/Trainium2 kernel for nn_BiLSTM_9028021256417.

Reference computation (see problem): 2-layer "bidirectional" LSTM where the
fw and bw chains are independent (no concat between layers), residual add on
the last layer, final output = (fw + bw) / 2.

Sharding (8 NeuronCores, SPMD — identical program, per-core data):
  cores 0-3: forward direction,  batch shards of 128
  cores 4-7: backward direction, batch shards of 128 (host feeds
             time-reversed x, so the device program is direction-agnostic)

Device layout: all state kept transposed —
  h, c           : [H=128 partitions, B=128 free]
  PSUM gate bank : [128, 4*B] with gate order (g, f, i, o) along free dim
  per-gate matmul: out[128, B] (+)= lhsT(W_g|U_g [128,128]).T @ rhs(x_t^T|h)
Matmul inputs are bf16 (1 cycle/row on the PE; fp32 would be 4), PSUM
accumulation and the cell state c stay fp32.

Schedule: layer 0 runs one timestep ahead of layer 1 in emission order,
so the per-engine in-order queues see [L0-chain(t+1), L1-chain(t)] every
body and neither layer's recurrence queues behind the other's tail.
W-projections (old operands) are emitted before the h-gated U matmuls in
each PSUM bank so a bank completes ~4 matmuls after h lands. Per layer
one sigmoid pair ([g] with bias 2s, [f,i,o] with bias s — per-gate bias
makes the rank-1 bias-fix matmuls unnecessary), cell state kept bf16 so
the whole i*g / f*c / add chain runs in DVE 2x mode; the residual
output add runs on GPSIMD off the critical path.
"""

import numpy as np
import ml_dtypes

import concourse.bass as bass
import concourse.tile as tile
from concourse import bacc, mybir
from concourse.bass_utils import run_bass_kernel_spmd

AF = mybir.ActivationFunctionType
FP32 = mybir.dt.float32
BF16 = mybir.dt.bfloat16
NP_BF16 = ml_dtypes.bfloat16

# Problem sizes (hardcoded per the harness contract).
B_TOT, T, E, H = 512, 200, 128, 128
NCORES = 8
NSHARD = 4          # batch shards per direction
B = B_TOT // NSHARD  # 128 per core
P = 128
NG = 4

# Device gate order (g, f, i, o) -> Keras 4H order is (i, f, g, o).
# keras slice index for each device gate slot:
KERAS_IDX = [2, 1, 0, 3]  # g, f, i, o
COL_G = slice(0 * B, 1 * B)
COL_F = slice(1 * B, 2 * B)
COL_I = slice(2 * B, 3 * B)
COL_O = slice(3 * B, 4 * B)


def _build_program(scalar_bias: float | None, t_steps: int = T):
    """Build the SPMD per-core Bass program (see module docstring)."""
    nc = bacc.Bacc("TRN2", target_bir_lowering=False, debug=False)

    xT = nc.dram_tensor("xT", [t_steps, E, B], BF16, kind="ExternalInput").ap()
    w = nc.dram_tensor("w", [2, NG, P, P], BF16, kind="ExternalInput").ap()
    u = nc.dram_tensor("u", [2, NG, P, P], BF16, kind="ExternalInput").ap()
    bias = nc.dram_tensor("bias", [2, NG, P, 1], FP32, kind="ExternalInput").ap()
    out = nc.dram_tensor("out", [t_steps, H, B], FP32, kind="ExternalOutput").ap()

    with tile.TileContext(nc) as tc:
        with (
            tc.tile_pool(name="wpool", bufs=1) as wpool,
            tc.tile_pool(name="xpool", bufs=8) as xpool,
            tc.tile_pool(name="zpool", bufs=4, space="PSUM") as zpool,
            tc.tile_pool(name="gpool", bufs=12) as gpool,
            tc.tile_pool(name="tpool", bufs=12) as tpool,
            tc.tile_pool(name="cpool", bufs=8) as cpool,
            tc.tile_pool(name="hpool", bufs=8) as hpool,
            tc.tile_pool(name="opool", bufs=8) as opool,
        ):
            w_t: dict = {}
            u_t: dict = {}
            b_t: dict = {}
            for l in range(2):
                for g in range(NG):
                    wt = wpool.tile([P, P], BF16, tag=f"w{l}{g}")
                    nc.sync.dma_start(wt[:], w[l, g])
                    w_t[l, g] = wt
                    ut = wpool.tile([P, P], BF16, tag=f"u{l}{g}")
                    nc.sync.dma_start(ut[:], u[l, g])
                    u_t[l, g] = ut
                    if scalar_bias is None:
                        bt = wpool.tile([P, 1], FP32, tag=f"b{l}{g}")
                        nc.sync.dma_start(bt[:], bias[l, g])
                        b_t[l, g] = bt

            if scalar_bias is not None:
                # per-partition bias AP for the g-gate (2*s: its weights are
                # host-scaled by 2 for the tanh-via-sigmoid trick)
                bg = wpool.tile([P, 1], FP32, tag="bg")
                nc.vector.memset(bg[:], 2.0 * float(scalar_bias))

            def dma_x(t):
                xt = xpool.tile([P, B], BF16, tag="xt")
                nc.sync.dma_start(xt[:], xT[t])
                return xt

            def emit_w0(t, xt):
                """x-projection matmuls for step t into a fresh z0 bank."""
                z0 = zpool.tile([P, NG * B], FP32, tag="z0")
                for g in range(NG):
                    nc.tensor.matmul(
                        z0[:, g * B : (g + 1) * B],
                        lhsT=w_t[0, g][:], rhs=xt[:],
                        start=(g == 0),
                        stop=(t == 0 and g == NG - 1),
                    )
                return z0

            def emit_u(l, z, h_prev, start, stop):
                for g in range(NG):
                    nc.tensor.matmul(
                        z[:, g * B : (g + 1) * B],
                        lhsT=u_t[l, g][:], rhs=h_prev[:],
                        start=(start and g == 0),
                        stop=(stop and g == NG - 1),
                    )

            def emit_w1(z1, h0, start, close):
                for g in range(NG):
                    nc.tensor.matmul(
                        z1[:, g * B : (g + 1) * B],
                        lhsT=w_t[1, g][:], rhs=h0[:],
                        start=(start and g == 0),
                        stop=(close and g == NG - 1),
                    )

            def gates_act(l, z):
                """Single fused sigmoid over all 4 gates (g pre-scaled)."""
                ys = gpool.tile([P, NG * B], BF16, tag=f"ys{l}")
                if scalar_bias is not None:
                    # g-gate weights are host-scaled by 2 (tanh via sigmoid),
                    # so its bias is 2*s; remaining gates get s. Emitting the
                    # g sigmoid first lets the s/i*g chain start early.
                    nc.scalar.activation(ys[:, COL_G], z[:, COL_G],
                                         AF.Sigmoid, bias=bg[:])
                    nc.scalar.activation(ys[:, B : NG * B], z[:, B : NG * B],
                                         AF.Sigmoid, bias=scalar_bias)
                else:
                    for g in range(NG):
                        nc.scalar.activation(
                            ys[:, g * B : (g + 1) * B],
                            z[:, g * B : (g + 1) * B],
                            AF.Sigmoid, bias=b_t[l, g][:],
                        )
                return ys

            def cupdate(l, ys, c_prev):
                """s = tanh(zg) = 2*sigmoid(2 zg)-1; c = f*c + i*s.
                f*c on GPSIMD (parallel with s/i*g on DVE)."""
                s = tpool.tile([P, B], BF16, tag=f"s{l}")
                nc.vector.tensor_scalar(
                    s[:], ys[:, COL_G], 2.0, -1.0,
                    mybir.AluOpType.mult, mybir.AluOpType.add,
                )
                t1 = tpool.tile([P, B], BF16, tag=f"t1{l}")
                nc.vector.tensor_mul(t1[:], ys[:, COL_I], s[:])  # i * tanh(g)
                if c_prev is None:
                    return t1
                t2 = tpool.tile([P, B], BF16, tag=f"t2{l}")
                nc.vector.tensor_mul(t2[:], ys[:, COL_F], c_prev[:])  # f * c
                c_new = cpool.tile([P, B], BF16, tag=f"c{l}")
                nc.vector.tensor_add(c_new[:], t1[:], t2[:])
                return c_new

            def hout(l, ys, c_new):
                tch = gpool.tile([P, B], BF16, tag=f"tc{l}")
                nc.scalar.activation(tch[:], c_new[:], AF.Tanh)
                h_new = hpool.tile([P, B], BF16, tag=f"h{l}")
                nc.vector.tensor_mul(h_new[:], ys[:, COL_O], tch[:])
                return h_new

            def emit_out(t, h1t, h0t):
                ot = opool.tile([P, B], FP32, tag="ot")
                nc.gpsimd.tensor_add(ot[:], h1t[:], h0t[:])
                nc.sync.dma_start(out[t], ot[:])

            # Software pipeline: layer 0 runs one timestep AHEAD of layer 1
            # in emission order, so per body the engine queues see
            # [L0-chain(t+1), L1-chain(t)] and layer 0's recurrence never
            # queues behind layer 1's tail. Exactly one data wait per body
            # on the PE (U0 on the fresh h0).
            c = {0: None, 1: None}
            h1_prev = None
            # Prologue: compute h0(0), bank z0(1) = W0(x1)+U0(h0(0)).
            x_tiles = {i: dma_x(i) for i in range(min(3, t_steps))}
            z0b = {0: emit_w0(0, x_tiles.pop(0))}
            ys0 = gates_act(0, z0b[0])
            c[0] = cupdate(0, ys0, None)
            h0_cur = hout(0, ys0, c[0])
            if t_steps > 1:
                z0b[1] = emit_w0(1, x_tiles.pop(1))
                emit_u(0, z0b[1], h0_cur, start=False, stop=True)
            for t in range(t_steps):
                # 1. prefetch + all PE work whose operands are one step old:
                #    W0(t+2) from x, layer-1 bank U1(h1(t-1)) + W1(h0(t)).
                if t + 3 < t_steps:
                    x_tiles[t + 3] = dma_x(t + 3)
                if t + 2 < t_steps:
                    z0b[t + 2] = emit_w0(t + 2, x_tiles.pop(t + 2))
                # W1 + bias fix first (operands old, stream early); the
                # h1-gated U1 matmuls last so the bank completes ~4 MMs
                # after h1 lands.
                z1 = zpool.tile([P, NG * B], FP32, tag="z1")
                emit_w1(z1, h0_cur, start=True,
                        close=(h1_prev is None))
                if h1_prev is not None:
                    # Dummy weight loads fill the wait for h1 so the PE
                    # never idles into a lower DVFS p-state; each is ~107ns
                    # and the real matmul reloads its own weights anyway.
                    for g in range(NG):
                        nc.tensor.ldweights(u_t[1, g][:])
                        nc.tensor.ldweights(w_t[0, g][:])
                    emit_u(1, z1, h1_prev, start=False, stop=True)
                # 2. both sigmoids back-to-back on the ACT queue
                ys0 = gates_act(0, z0b.pop(t + 1)) if t + 1 < t_steps else None
                ys1 = gates_act(1, z1)
                # 3. full layer-0 tail first (cell, tanh, h, U0) so layer-1's
                #    DVE ops never sit between h0 and the critical U0 matmuls
                h0_next = None
                if ys0 is not None:
                    c[0] = cupdate(0, ys0, c[0])
                    h0_next = hout(0, ys0, c[0])
                    if t + 2 < t_steps:
                        emit_u(0, z0b[t + 2], h0_next, start=False, stop=True)
                # 4. layer-1 cell/h, residual output
                c[1] = cupdate(1, ys1, c[1])
                h1 = hout(1, ys1, c[1])
                emit_out(t, h1, h0_cur)
                h1_prev = h1
                if h0_next is not None:
                    h0_cur = h0_next

    nc.compile()
    return nc


_PROGRAM_CACHE: dict = {}


def _get_program(scalar_bias, t_steps: int = T):
    key = (scalar_bias, t_steps)
    if key not in _PROGRAM_CACHE:
        _PROGRAM_CACHE[key] = _build_program(scalar_bias, t_steps)
    return _PROGRAM_CACHE[key]


def _prep_inputs(x, W, U, b, scalar_bias):
    """Build the 8 per-core input maps."""
    in_maps = []
    per_dir = {}
    for d in range(2):
        wd = np.empty((2, NG, P, P), dtype=NP_BF16)
        ud = np.empty((2, NG, P, P), dtype=NP_BF16)
        bd = np.empty((2, NG, P, 1), dtype=np.float32)
        for l in range(2):
            for g in range(NG):
                ks = KERAS_IDX[g]
                # device gate slot 0 is the candidate gate, computed as
                # tanh(zg) = 2*sigmoid(2*zg) - 1: scale weights/bias by 2
                sc = 2.0 if g == 0 else 1.0
                wd[l, g] = (sc * W[l, d][:, ks * H : (ks + 1) * H]).astype(NP_BF16)
                ud[l, g] = (sc * U[l, d][:, ks * H : (ks + 1) * H]).astype(NP_BF16)
                bd[l, g, :, 0] = (sc * b[l, d][ks * H : (ks + 1) * H]).astype(np.float32)
        per_dir[d] = (wd, ud, bd)

    for core in range(NCORES):
        d = core // NSHARD
        s = core % NSHARD
        xs = x[s * B : (s + 1) * B]           # [B, T, E]
        if d == 1:
            xs = xs[:, ::-1, :]               # time-reverse for backward dir
        xTc = np.ascontiguousarray(np.transpose(xs, (1, 2, 0))).astype(NP_BF16)
        wd, ud, bd = per_dir[d]
        in_maps.append({"xT": xTc, "w": wd, "u": ud, "bias": bd})
    return in_maps


def _postprocess(results, dtype):
    full = np.empty((B_TOT, T, H), dtype=np.float32)
    for s in range(NSHARD):
        fw = np.asarray(results[s]["out"])            # [T, H, B]
        bw = np.asarray(results[NSHARD + s]["out"])   # [T, H, B] (reversed time)
        fw_b = np.transpose(fw, (2, 0, 1))            # [B, T, H]
        bw_b = np.transpose(bw, (2, 0, 1))[:, ::-1, :]
        full[s * B : (s + 1) * B] = (fw_b + bw_b) * 0.5
    return full.astype(dtype)


def run(x, W, U, b, **spmd_kwargs):
    """Run the kernel; returns (output, BassKernelResults)."""
    x = np.asarray(x)
    W = np.asarray(W)
    U = np.asarray(U)
    b = np.asarray(b)
    b0 = float(np.asarray(b).flat[0])
    scalar_bias = b0 if np.all(b == b0) else None
    nc = _get_program(scalar_bias)
    in_maps = _prep_inputs(x, W, U, b, scalar_bias)
    res = run_bass_kernel_spmd(nc, in_maps, core_ids=list(range(NCORES)), **spmd_kwargs)
    out = _postprocess(res.results, x.dtype)
    return out, res


def kernel(x, W, U, b):
    out, _ = run(x, W, U, b)
    return out


# revision 45
# speedup vs baseline: 1.0942x; 1.0942x over previous
"""Bass/Trainium2 kernel for nn_BiLSTM_9028021256417.

Reference computation (see problem): 2-layer "bidirectional" LSTM where the
fw and bw chains are independent (no concat between layers), residual add on
the last layer, final output = (fw + bw) / 2.

Sharding (8 NeuronCores, SPMD — identical program, per-core data):
  cores 0-3: forward direction,  batch shards of 128
  cores 4-7: backward direction, batch shards of 128 (host feeds
             time-reversed x, so the device program is direction-agnostic)

Device layout: all state kept transposed —
  h, c           : [H=128 partitions, B=128 free]
  PSUM gate bank : [128, 4*B] with gate order (g, f, i, o) along free dim
  per-gate matmul: out[128, B] (+)= lhsT(W_g|U_g [128,128]).T @ rhs(x_t^T|h)
Matmul inputs are bf16 (1 cycle/row on the PE; fp32 would be 4), PSUM
accumulation and the cell state c stay fp32.

Schedule: layer 0 runs one timestep ahead of layer 1 in emission order,
so the per-engine in-order queues see [L0-chain(t+1), L1-chain(t)] every
body and neither layer's recurrence queues behind the other's tail.
W-projections (old operands) are emitted before the h-gated U matmuls in
each PSUM bank so a bank completes ~4 matmuls after h lands. Per layer
one sigmoid pair ([g] with bias 2s, [f,i,o] with bias s — per-gate bias
makes the rank-1 bias-fix matmuls unnecessary), cell state kept bf16 so
the whole i*g / f*c / add chain runs in DVE 2x mode; the residual
output add runs on GPSIMD off the critical path.
"""

import numpy as np
import ml_dtypes

import concourse.bass as bass
import concourse.tile as tile
from concourse import bacc, mybir
from concourse.bass_utils import run_bass_kernel_spmd

AF = mybir.ActivationFunctionType
FP32 = mybir.dt.float32
BF16 = mybir.dt.bfloat16
NP_BF16 = ml_dtypes.bfloat16

# Problem sizes (hardcoded per the harness contract).
B_TOT, T, E, H = 512, 200, 128, 128
NCORES = 8
NSHARD = 4          # batch shards per direction
B = B_TOT // NSHARD  # 128 per core
P = 128
NG = 4

# Device gate order (g, f, i, o) -> Keras 4H order is (i, f, g, o).
# keras slice index for each device gate slot:
KERAS_IDX = [2, 1, 0, 3]  # g, f, i, o
COL_G = slice(0 * B, 1 * B)
COL_F = slice(1 * B, 2 * B)
COL_I = slice(2 * B, 3 * B)
COL_O = slice(3 * B, 4 * B)


def _build_program(scalar_bias: float | None, t_steps: int = T):
    """Build the SPMD per-core Bass program (see module docstring)."""
    nc = bacc.Bacc("TRN2", target_bir_lowering=False, debug=False)

    xT = nc.dram_tensor("xT", [t_steps, E, B], BF16, kind="ExternalInput").ap()
    w = nc.dram_tensor("w", [2, NG, P, P], BF16, kind="ExternalInput").ap()
    u = nc.dram_tensor("u", [2, NG, P, P], BF16, kind="ExternalInput").ap()
    bias = nc.dram_tensor("bias", [2, NG, P, 1], FP32, kind="ExternalInput").ap()
    out = nc.dram_tensor("out", [t_steps, H, B], FP32, kind="ExternalOutput").ap()

    with tile.TileContext(nc) as tc:
        with (
            tc.tile_pool(name="wpool", bufs=1) as wpool,
            tc.tile_pool(name="xpool", bufs=8) as xpool,
            tc.tile_pool(name="zpool", bufs=4, space="PSUM") as zpool,
            tc.tile_pool(name="gpool", bufs=12) as gpool,
            tc.tile_pool(name="tpool", bufs=12) as tpool,
            tc.tile_pool(name="cpool", bufs=8) as cpool,
            tc.tile_pool(name="hpool", bufs=8) as hpool,
            tc.tile_pool(name="opool", bufs=8) as opool,
        ):
            w_t: dict = {}
            u_t: dict = {}
            b_t: dict = {}
            for l in range(2):
                for g in range(NG):
                    wt = wpool.tile([P, P], BF16, tag=f"w{l}{g}")
                    nc.sync.dma_start(wt[:], w[l, g])
                    w_t[l, g] = wt
                    ut = wpool.tile([P, P], BF16, tag=f"u{l}{g}")
                    nc.sync.dma_start(ut[:], u[l, g])
                    u_t[l, g] = ut
                    if scalar_bias is None:
                        bt = wpool.tile([P, 1], FP32, tag=f"b{l}{g}")
                        nc.sync.dma_start(bt[:], bias[l, g])
                        b_t[l, g] = bt

            if scalar_bias is not None:
                # per-partition bias AP for the g-gate (2*s: its weights are
                # host-scaled by 2 for the tanh-via-sigmoid trick)
                bg = wpool.tile([P, 1], FP32, tag="bg")
                nc.vector.memset(bg[:], 2.0 * float(scalar_bias))

            def dma_x(t):
                xt = xpool.tile([P, B], BF16, tag="xt")
                nc.sync.dma_start(xt[:], xT[t])
                return xt

            def emit_w0(t, xt):
                """x-projection matmuls for step t into a fresh z0 bank."""
                z0 = zpool.tile([P, NG * B], FP32, tag="z0")
                for g in range(NG):
                    nc.tensor.matmul(
                        z0[:, g * B : (g + 1) * B],
                        lhsT=w_t[0, g][:], rhs=xt[:],
                        start=(g == 0),
                        stop=(t == 0 and g == NG - 1),
                    )
                return z0

            def emit_u(l, z, h_prev, start, stop):
                for g in range(NG):
                    nc.tensor.matmul(
                        z[:, g * B : (g + 1) * B],
                        lhsT=u_t[l, g][:], rhs=h_prev[:],
                        start=(start and g == 0),
                        stop=(stop and g == NG - 1),
                    )

            def emit_w1(z1, h0, start, close):
                for g in range(NG):
                    nc.tensor.matmul(
                        z1[:, g * B : (g + 1) * B],
                        lhsT=w_t[1, g][:], rhs=h0[:],
                        start=(start and g == 0),
                        stop=(close and g == NG - 1),
                    )

            def gates_act(l, z):
                """Single fused sigmoid over all 4 gates (g pre-scaled)."""
                ys = gpool.tile([P, NG * B], BF16, tag=f"ys{l}")
                if scalar_bias is not None:
                    # g-gate weights are host-scaled by 2 (tanh via sigmoid),
                    # so its bias is 2*s; remaining gates get s. Emitting the
                    # g sigmoid first lets the s/i*g chain start early.
                    nc.scalar.activation(ys[:, COL_G], z[:, COL_G],
                                         AF.Sigmoid, bias=bg[:])
                    nc.scalar.activation(ys[:, B : NG * B], z[:, B : NG * B],
                                         AF.Sigmoid, bias=scalar_bias)
                else:
                    for g in range(NG):
                        nc.scalar.activation(
                            ys[:, g * B : (g + 1) * B],
                            z[:, g * B : (g + 1) * B],
                            AF.Sigmoid, bias=b_t[l, g][:],
                        )
                return ys

            def cupdate(l, ys, c_prev):
                """s = tanh(zg) = 2*sigmoid(2 zg)-1; c = f*c + i*s.
                f*c on GPSIMD (parallel with s/i*g on DVE)."""
                s = tpool.tile([P, B], BF16, tag=f"s{l}")
                nc.vector.tensor_scalar(
                    s[:], ys[:, COL_G], 2.0, -1.0,
                    mybir.AluOpType.mult, mybir.AluOpType.add,
                )
                t1 = tpool.tile([P, B], BF16, tag=f"t1{l}")
                nc.vector.tensor_mul(t1[:], ys[:, COL_I], s[:])  # i * tanh(g)
                if c_prev is None:
                    return t1
                t2 = tpool.tile([P, B], BF16, tag=f"t2{l}")
                nc.vector.tensor_mul(t2[:], ys[:, COL_F], c_prev[:])  # f * c
                c_new = cpool.tile([P, B], BF16, tag=f"c{l}")
                nc.vector.tensor_add(c_new[:], t1[:], t2[:])
                return c_new

            def hout(l, ys, c_new):
                tch = gpool.tile([P, B], BF16, tag=f"tc{l}")
                nc.scalar.activation(tch[:], c_new[:], AF.Tanh)
                h_new = hpool.tile([P, B], BF16, tag=f"h{l}")
                nc.vector.tensor_mul(h_new[:], ys[:, COL_O], tch[:])
                return h_new

            def emit_out(t, h1t, h0t):
                ot = opool.tile([P, B], FP32, tag="ot")
                nc.gpsimd.tensor_add(ot[:], h1t[:], h0t[:])
                nc.sync.dma_start(out[t], ot[:])

            # Software pipeline: layer 0 runs one timestep AHEAD of layer 1
            # in emission order, so per body the engine queues see
            # [L0-chain(t+1), L1-chain(t)] and layer 0's recurrence never
            # queues behind layer 1's tail. Exactly one data wait per body
            # on the PE (U0 on the fresh h0).
            c = {0: None, 1: None}
            h1_prev = None
            # Prologue: compute h0(0), bank z0(1) = W0(x1)+U0(h0(0)).
            x_tiles = {i: dma_x(i) for i in range(min(3, t_steps))}
            z0b = {0: emit_w0(0, x_tiles.pop(0))}
            ys0 = gates_act(0, z0b[0])
            c[0] = cupdate(0, ys0, None)
            h0_cur = hout(0, ys0, c[0])
            if t_steps > 1:
                z0b[1] = emit_w0(1, x_tiles.pop(1))
                emit_u(0, z0b[1], h0_cur, start=False, stop=True)
            for t in range(t_steps):
                # 1. prefetch + all PE work whose operands are one step old:
                #    W0(t+2) from x, layer-1 bank U1(h1(t-1)) + W1(h0(t)).
                if t + 3 < t_steps:
                    x_tiles[t + 3] = dma_x(t + 3)
                if t + 2 < t_steps:
                    z0b[t + 2] = emit_w0(t + 2, x_tiles.pop(t + 2))
                # W1 + bias fix first (operands old, stream early); the
                # h1-gated U1 matmuls last so the bank completes ~4 MMs
                # after h1 lands.
                z1 = zpool.tile([P, NG * B], FP32, tag="z1")
                emit_w1(z1, h0_cur, start=True,
                        close=(h1_prev is None))
                if h1_prev is not None:
                    emit_u(1, z1, h1_prev, start=False, stop=True)
                # 2. both sigmoids back-to-back on the ACT queue
                ys0 = gates_act(0, z0b.pop(t + 1)) if t + 1 < t_steps else None
                ys1 = gates_act(1, z1)
                # 3. full layer-0 tail first (cell, tanh, h, U0) so layer-1's
                #    DVE ops never sit between h0 and the critical U0 matmuls
                h0_next = None
                if ys0 is not None:
                    c[0] = cupdate(0, ys0, c[0])
                    h0_next = hout(0, ys0, c[0])
                    if t + 2 < t_steps:
                        emit_u(0, z0b[t + 2], h0_next, start=False, stop=True)
                # 4. layer-1 cell/h, residual output
                c[1] = cupdate(1, ys1, c[1])
                h1 = hout(1, ys1, c[1])
                emit_out(t, h1, h0_cur)
                h1_prev = h1
                if h0_next is not None:
                    h0_cur = h0_next

    nc.compile()
    return nc


_PROGRAM_CACHE: dict = {}


def _get_program(scalar_bias, t_steps: int = T):
    key = (scalar_bias, t_steps)
    if key not in _PROGRAM_CACHE:
        _PROGRAM_CACHE[key] = _build_program(scalar_bias, t_steps)
    return _PROGRAM_CACHE[key]


def _prep_inputs(x, W, U, b, scalar_bias):
    """Build the 8 per-core input maps."""
    in_maps = []
    per_dir = {}
    for d in range(2):
        wd = np.empty((2, NG, P, P), dtype=NP_BF16)
        ud = np.empty((2, NG, P, P), dtype=NP_BF16)
        bd = np.empty((2, NG, P, 1), dtype=np.float32)
        for l in range(2):
            for g in range(NG):
                ks = KERAS_IDX[g]
                # device gate slot 0 is the candidate gate, computed as
                # tanh(zg) = 2*sigmoid(2*zg) - 1: scale weights/bias by 2
                sc = 2.0 if g == 0 else 1.0
                wd[l, g] = (sc * W[l, d][:, ks * H : (ks + 1) * H]).astype(NP_BF16)
                ud[l, g] = (sc * U[l, d][:, ks * H : (ks + 1) * H]).astype(NP_BF16)
                bd[l, g, :, 0] = (sc * b[l, d][ks * H : (ks + 1) * H]).astype(np.float32)
        per_dir[d] = (wd, ud, bd)

    for core in range(NCORES):
        d = core // NSHARD
        s = core % NSHARD
        xs = x[s * B : (s + 1) * B]           # [B, T, E]
        if d == 1:
            xs = xs[:, ::-1, :]               # time-reverse for backward dir
        xTc = np.ascontiguousarray(np.transpose(xs, (1, 2, 0))).astype(NP_BF16)
        wd, ud, bd = per_dir[d]
        in_maps.append({"xT": xTc, "w": wd, "u": ud, "bias": bd})
    return in_maps


def _postprocess(results, dtype):
    full = np.empty((B_TOT, T, H), dtype=np.float32)
    for s in range(NSHARD):
        fw = np.asarray(results[s]["out"])            # [T, H, B]
        bw = np.asarray(results[NSHARD + s]["out"])   # [T, H, B] (reversed time)
        fw_b = np.transpose(fw, (2, 0, 1))            # [B, T, H]
        bw_b = np.transpose(bw, (2, 0, 1))[:, ::-1, :]
        full[s * B : (s + 1) * B] = (fw_b + bw_b) * 0.5
    return full.astype(dtype)


def run(x, W, U, b, **spmd_kwargs):
    """Run the kernel; returns (output, BassKernelResults)."""
    x = np.asarray(x)
    W = np.asarray(W)
    U = np.asarray(U)
    b = np.asarray(b)
    b0 = float(np.asarray(b).flat[0])
    scalar_bias = b0 if np.all(b == b0) else None
    nc = _get_program(scalar_bias)
    in_maps = _prep_inputs(x, W, U, b, scalar_bias)
    res = run_bass_kernel_spmd(nc, in_maps, core_ids=list(range(NCORES)), **spmd_kwargs)
    out = _postprocess(res.results, x.dtype)
    return out, res


def kernel(x, W, U, b):
    out, _ = run(x, W, U, b)
    return out
